# revision 15
# baseline (speedup 1.0000x reference)
"""BiMamba Trainium2 kernel (8 NeuronCores, SPMD).

Sharding: core = dir(2) x batch(2) x d_inner-half(2).
Each core runs one direction's mamba block on one batch element for half of
d_inner. The xproj (which contracts over full d_inner) is handled by having
every core compute the full xi/conv/silu (cheap duplication) so no cross-core
communication is needed. The final out-proj + concat + output projection are
algebraically folded into one matmul with W_eff = proj_W[:, dir] @ out_W_dir;
each core emits a partial (d_model, L) which the host sums across the 4 cores
of each batch element.

v3 engine plan (from trace + microbench):
- depthwise conv on PE: 4 diagonal-matrix matmuls into PSUM; in-proj chunks
  carry a 3-column overlap so no halo copies are needed.
- selective scan: native tensor_tensor_scan on DVE, chained over two
  time-halves so phase B's first half overlaps phase A's last chunks
  (emission interleaved; engine streams are in-order).
- dA = exp(-n*dt) on the scalar engine.
- d1 = bsc*B always on DVE (it feeds the scan); ch = h*C mostly on GpSimd.
- y2 accumulation over the 16 states via GpSimd-issued accumulate-DMAs
  (SBUF->SBUF bf16) running on the DMA engines.
"""

import sys

sys.path.insert(0, "/opt/trn_rl_repo")

import numpy as np
import ml_dtypes

import concourse.bass as bass
import concourse.bacc as bacc
import concourse.mybir as mybir
import concourse.tile as tile
from concourse import bass_utils

F32 = mybir.dt.float32
BF16 = mybir.dt.bfloat16
AF = mybir.ActivationFunctionType
ALU = mybir.AluOpType

B, L, DM = 2, 2048, 1024
DI = 2048            # d_inner
DH = DI // 2         # per-core half of d_inner
N = 16               # d_state
R = 64               # dt_rank
K4 = 4               # d_conv
TC = 512             # time chunk for matmul phases
NCHUNK = L // TC
NBLK_DM = DM // 128      # 8 k-blocks over d_model
NBLK_DH = DH // 128      # 8 blocks over own half
NBLK_DF = DI // 128      # 16 blocks over full d_inner
LH = L // 2              # phase-B half length

_CACHED = {}


def _build_module():
    nc = bacc.Bacc("TRN2", target_bir_lowering=False, debug=False, num_devices=8)

    def din(name, shape, dt):
        return nc.dram_tensor(name, list(shape), dt, kind="ExternalInput").ap()

    xT = din("xT", (DM, L), BF16)                 # x (possibly flipped).T
    w_in = din("w_in", (DM, DI + DH), BF16)       # lhsT: [xi_own|xi_oth|z_own]
    w_xp = din("w_xp", (DI, 2 * N + R), BF16)     # lhsT for xproj (rows reordered)
    w_dt = din("w_dt", (R, DH), BF16)             # lhsT for dt proj (own half)
    w_out = din("w_out", (DH, DM), BF16)          # lhsT: W_eff own-half rows
    conv_diag = din("conv_diag", (NBLK_DF * K4 * 128, 128), BF16)  # diag conv mats
    conv_b = din("conv_b", (DI, 1), F32)
    dt_b = din("dt_b", (DH, 1), F32)
    Dv = din("Dv", (DH, 1), F32)
    out_d = nc.dram_tensor("out", [DM, L], F32, kind="ExternalOutput").ap()
    z_spill = nc.dram_tensor("z_spill", [DH, L], BF16, kind="Internal").ap()
    xc_spill = nc.dram_tensor("xc_spill", [DH, L], BF16, kind="Internal").ap()
    bc_spill = nc.dram_tensor("bc_spill", [2 * N, L], BF16, kind="Internal").ap()
    y2_spill = nc.dram_tensor("y2_spill", [DH, L], BF16, kind="Internal").ap()

    with tile.TileContext(nc) as tc:
        _emit(nc, tc, xT, w_in, w_xp, w_dt, w_out, conv_diag, conv_b, dt_b, Dv,
              out_d, z_spill, xc_spill, bc_spill, y2_spill)
    nc.compile()
    return nc


def _emit(nc, tc, xT, w_in, w_xp, w_dt, w_out, conv_diag, conv_b, dt_b, Dv,
          out_d, z_spill, xc_spill, bc_spill, y2_spill):
    from contextlib import ExitStack
    ctx = ExitStack()
    with ctx:
        # ---------------- persistent weights/consts ----------------
        wpool = ctx.enter_context(tc.tile_pool(name="weights", bufs=1))
        conv_b_sb = wpool.tile([128, NBLK_DF], F32, tag="conv_b", name="conv_b")
        nc.sync.dma_start(conv_b_sb[:],
                          conv_b.rearrange("(k p) c -> p k c", p=128))
        dt_b_sb = wpool.tile([128, NBLK_DH], F32, tag="dt_b", name="dt_b")
        nc.sync.dma_start(dt_b_sb[:],
                          dt_b.rearrange("(k p) c -> p k c", p=128))
        Dv_sb = wpool.tile([128, NBLK_DH], F32, tag="Dv", name="Dv")
        nc.sync.dma_start(Dv_sb[:],
                          Dv.rearrange("(k p) c -> p k c", p=128))

        # ---------------- resident activations ----------------
        rpool = ctx.enter_context(tc.tile_pool(name="resident", bufs=1))
        dt_own = [rpool.tile([128, L], BF16, tag=f"dt{b}", name=f"dt{b}")
                  for b in range(NBLK_DH)]
        bsc = [rpool.tile([128, L], BF16, tag=f"bsc{b}", name=f"bsc{b}")
               for b in range(NBLK_DH)]
        # chunk-boundary scan states: one [128, 1] column per (n, b)
        hs_pool = ctx.enter_context(tc.tile_pool(name="hstate", bufs=1))
        hstate = hs_pool.tile([128, N * NBLK_DH], F32, tag="hstate", name="hstate")

        # phase-B rotating pool must outlive (so open before) the phase-A pools
        bpool = ctx.enter_context(tc.tile_pool(name="phaseB", bufs=2))

        # ================= Phase A emitters =================
        actx = ExitStack()
        apw = actx.enter_context(tc.tile_pool(name="phaseA_w", bufs=1))
        apool = actx.enter_context(tc.tile_pool(name="phaseA", bufs=1))
        apsum = actx.enter_context(tc.tile_pool(name="phaseA_ps", bufs=2,
                                                space="PSUM"))
        cpsum = actx.enter_context(tc.tile_pool(name="phaseA_cps", bufs=2,
                                                space="PSUM"))
        ppsum = actx.enter_context(tc.tile_pool(name="phaseA_pps", bufs=1,
                                                space="PSUM"))
        mpool = actx.enter_context(tc.tile_pool(name="phaseA_misc", bufs=1))
        xi_sb = [mpool.tile([128, 3 + TC], BF16, tag=f"xi{m}", name=f"xi{m}")
                 for m in range(NBLK_DF)]

        w_in_sb = []
        for k in range(NBLK_DM):
            t = apw.tile([128, DI + DH], BF16, tag=f"w_in{k}", name=f"w_in{k}")
            nc.sync.dma_start(t[:], w_in[k * 128:(k + 1) * 128, :])
            w_in_sb.append(t)
        w_xp_sb = []
        for k in range(NBLK_DF):
            t = apw.tile([128, 2 * N + R], BF16, tag=f"w_xp{k}", name=f"w_xp{k}")
            nc.sync.dma_start(t[:], w_xp[k * 128:(k + 1) * 128, :])
            w_xp_sb.append(t)
        w_dt_sb = apw.tile([R, DH], BF16, tag="w_dt", name="w_dt")
        nc.sync.dma_start(w_dt_sb[:], w_dt[:, :])

        def emit_A_chunk(c):
            t0 = c * TC
            x_sb = []
            for k in range(NBLK_DM):
                t = apool.tile([128, TC], BF16, tag=f"x{k}", name=f"x{k}")
                nc.sync.dma_start(t[:],
                                  xT[k * 128:(k + 1) * 128, t0:t0 + TC])
                x_sb.append(t)
            xc_chunk = []
            for m in range(NBLK_DF + NBLK_DH):   # 16 xi blocks + 8 z blocks
                if m < NBLK_DF:
                    ps = apsum.tile([128, TC], F32, tag="inproj", name="inproj")
                    for k in range(NBLK_DM):
                        nc.tensor.matmul(ps[:],
                                         w_in_sb[k][:, m * 128:(m + 1) * 128],
                                         x_sb[k][:], start=(k == 0),
                                         stop=(k == NBLK_DM - 1))
                    xi = xi_sb[m]
                    if c == 0:
                        nc.vector.memset(xi[:, 0:3], 0.0)
                    else:
                        # save last 3 cols of previous chunk as the new halo
                        nc.scalar.activation(xi[:, 0:3], xi[:, TC:TC + 3], AF.Copy)
                    nc.scalar.activation(xi[:, 3:3 + TC], ps[:], AF.Copy)
                    cdg = apool.tile([128, K4 * 128], BF16, tag="cdiag",
                                     name="cdiag")
                    nc.sync.dma_start(
                        cdg[:],
                        conv_diag[m * K4 * 128:(m + 1) * K4 * 128, :]
                        .rearrange("(q p) j -> p q j", p=128))
                    cps = cpsum.tile([128, TC], F32, tag="convps", name="convps")
                    for kk in range(K4):
                        nc.tensor.matmul(cps[:],
                                         cdg[:, kk * 128:(kk + 1) * 128],
                                         xi[:, kk:kk + TC],
                                         start=(kk == 0), stop=(kk == K4 - 1))
                    xc_t = mpool.tile([128, TC], BF16, tag=f"xct{m}",
                                      name=f"xct{m}")
                    nc.scalar.activation(xc_t[:], cps[:], AF.Silu,
                                         bias=conv_b_sb[:, m:m + 1])
                    if m < NBLK_DH:
                        nc.sync.dma_start(
                            xc_spill[m * 128:(m + 1) * 128, t0:t0 + TC],
                            xc_t[:])
                    xc_chunk.append(xc_t)
                else:
                    ps = apsum.tile([128, TC], F32, tag="inproj", name="inproj")
                    for k in range(NBLK_DM):
                        nc.tensor.matmul(ps[:],
                                         w_in_sb[k][:, m * 128:(m + 1) * 128],
                                         x_sb[k][:], start=(k == 0),
                                         stop=(k == NBLK_DM - 1))
                    zb = m - NBLK_DF
                    zt = apool.tile([128, TC], BF16, tag="zt", name="zt")
                    nc.scalar.activation(zt[:], ps[:], AF.Silu)
                    nc.sync.dma_start(
                        z_spill[zb * 128:(zb + 1) * 128, t0:t0 + TC], zt[:])

            # xproj: (2N+R, TC) accumulated over 16 channel blocks
            ps96 = ppsum.tile([R + 2 * N, TC], F32, tag="xproj", name="xproj")
            for k in range(NBLK_DF):
                nc.tensor.matmul(ps96[:], w_xp_sb[k][:], xc_chunk[k][:],
                                 start=(k == 0), stop=(k == NBLK_DF - 1))
            xdbl = apool.tile([R + 2 * N, TC], BF16, tag="xdbl", name="xdbl")
            nc.scalar.activation(xdbl[:], ps96[:], AF.Copy)
            # B and C rows -> DRAM (bf16) for later broadcast-reload
            bcc = apool.tile([2 * N, TC], BF16, tag="bcc", name="bcc")
            nc.vector.tensor_copy(bcc[:], xdbl[R:R + 2 * N, :])
            nc.sync.dma_start(bc_spill[:, t0:t0 + TC], bcc[:])
            # dt proj + softplus, then bsc = dt * xc
            for mb in range(NBLK_DH):
                psd = ppsum.tile([128, TC], F32, tag="dtproj", name="dtproj")
                nc.tensor.matmul(psd[:], w_dt_sb[:, mb * 128:(mb + 1) * 128],
                                 xdbl[0:R, :], start=True, stop=True)
                spe = apool.tile([128, TC], F32, tag="spe", name="spe")
                nc.scalar.activation(spe[:], psd[:], AF.Exp,
                                     bias=dt_b_sb[:, mb:mb + 1])
                nc.scalar.activation(dt_own[mb][:, t0:t0 + TC], spe[:],
                                     AF.Ln, bias=1.0)
                nc.vector.tensor_tensor(bsc[mb][:, t0:t0 + TC],
                                        dt_own[mb][:, t0:t0 + TC],
                                        xc_chunk[mb][:], ALU.mult)

        # ================= Phase B emitters =================
        # static engine assignment for ch = h*C between DVE and GpSimd.
        # Measured per-half costs: DVE ~1.08us, GpSimd ~4.4us; DVE starts with
        # the scans+d1 (~860us) vs GpSimd's accum-DMA preps (~240us).
        ch_on_v = set()
        vload, gload = 860.0, 243.0
        for i in range(N * NBLK_DH):
            if vload + 2 * 1.08 <= gload + 2 * 4.37:
                ch_on_v.add(i)
                vload += 2 * 1.08
            else:
                gload += 2 * 4.37

        def emit_B_half(hf, n):
            lo = hf * LH
            B_bc = bpool.tile([128, LH], BF16, tag="B_bc", name="B_bc")
            C_bc = bpool.tile([128, LH], BF16, tag="C_bc", name="C_bc")
            nc.sync.dma_start(
                B_bc[:], bc_spill[n:n + 1, lo:lo + LH].partition_broadcast(128))
            nc.sync.dma_start(
                C_bc[:], bc_spill[N + n:N + n + 1, lo:lo + LH]
                .partition_broadcast(128))
            for b in range(NBLK_DH):
                dA = bpool.tile([128, LH], BF16, tag="dA", name="dA")
                nc.scalar.activation(dA[:], dt_own[b][:, lo:lo + LH], AF.Exp,
                                     scale=-float(n + 1))
                d1 = bpool.tile([128, LH], BF16, tag="d1", name="d1")
                nc.vector.tensor_tensor(d1[:], bsc[b][:, lo:lo + LH], B_bc[:],
                                        ALU.mult)
                h = bpool.tile([128, LH], BF16, tag="h", name="h")
                sc = n * NBLK_DH + b
                if hf == 0:
                    nc.vector.tensor_tensor_scan(h[:], dA[:], d1[:], 0.0,
                                                 ALU.mult, ALU.add)
                    nc.scalar.activation(hstate[:, sc:sc + 1],
                                         h[:, LH - 1:LH], AF.Copy)
                else:
                    nc.vector.tensor_tensor_scan(h[:], dA[:], d1[:],
                                                 hstate[:, sc:sc + 1],
                                                 ALU.mult, ALU.add)
                ch = bpool.tile([128, LH], BF16, tag="ch", name="ch")
                if sc in ch_on_v:
                    nc.vector.tensor_tensor(ch[:], h[:], C_bc[:], ALU.mult)
                else:
                    nc.gpsimd.tensor_tensor(ch[:], h[:], C_bc[:], ALU.mult)
                dst = y2_spill[b * 128:(b + 1) * 128, lo:lo + LH]
                if n == 0:
                    nc.sync.dma_start(dst, ch[:])
                else:
                    nc.gpsimd.dma_start(dst, ch[:], accum_op=ALU.add)

        # ================= emission: interleave A and B1 =================
        emit_A_chunk(0)
        emit_A_chunk(1)
        for n in range(N):
            if n == 4:
                emit_A_chunk(2)
            if n == 9:
                emit_A_chunk(3)
            emit_B_half(0, n)
        actx.close()
        for n in range(N):
            emit_B_half(1, n)

        # ============= Phase C: gate + out-proj =============
        with tc.tile_pool(name="phaseC", bufs=2) as cpool, \
             tc.tile_pool(name="phaseC_ps", bufs=2, space="PSUM") as cpsum2, \
             tc.tile_pool(name="phaseC_s", bufs=1) as spool, \
             tc.tile_pool(name="phaseC_w", bufs=1) as cwpool:
            w_out_sb = []
            for k in range(NBLK_DH):
                t = cwpool.tile([128, DM], BF16, tag=f"w_out{k}", name=f"w_out{k}")
                nc.sync.dma_start(t[:], w_out[k * 128:(k + 1) * 128, :])
                w_out_sb.append(t)
            s_sb = []
            for b in range(NBLK_DH):
                xcr = cpool.tile([128, L], BF16, tag="xcr", name="xcr")
                nc.sync.dma_start(xcr[:], xc_spill[b * 128:(b + 1) * 128, :])
                zs = cpool.tile([128, L], BF16, tag="zs", name="zs")
                nc.sync.dma_start(zs[:], z_spill[b * 128:(b + 1) * 128, :])
                y2r = cpool.tile([128, L], BF16, tag="y2r", name="y2r")
                nc.sync.dma_start(y2r[:], y2_spill[b * 128:(b + 1) * 128, :])
                s = spool.tile([128, L], BF16, tag=f"s{b}", name=f"s{b}")
                # s = xcr*D + y2  (fused), then s *= silu(z)
                nc.vector.scalar_tensor_tensor(s[:], xcr[:], Dv_sb[:, b:b + 1],
                                               y2r[:], ALU.mult, ALU.add)
                nc.vector.tensor_tensor(s[:], s[:], zs[:], ALU.mult)
                s_sb.append(s)
            for m in range(NBLK_DM):
                for c in range(NCHUNK):
                    ps = cpsum2.tile([128, TC], F32, tag="oproj", name="oproj")
                    for k in range(NBLK_DH):
                        nc.tensor.matmul(
                            ps[:], w_out_sb[k][:, m * 128:(m + 1) * 128],
                            s_sb[k][:, c * TC:(c + 1) * TC],
                            start=(k == 0), stop=(k == NBLK_DH - 1))
                    ot = cpool.tile([128, TC], F32, tag="ot", name="ot")
                    nc.vector.tensor_copy(ot[:], ps[:])
                    nc.sync.dma_start(
                        out_d[m * 128:(m + 1) * 128, c * TC:(c + 1) * TC],
                        ot[:])


def _prep_inputs(inputs):
    """Build the 8 per-core input maps from full inputs (numpy fp32)."""
    bf = ml_dtypes.bfloat16
    x = np.asarray(inputs["x"], np.float32)
    maps = []
    for core in range(8):
        dire, bat, half = core // 4, (core // 2) % 2, core % 2
        p = "fwd" if dire == 0 else "bwd"
        in_W = np.asarray(inputs[p + "_in_W"], np.float32)
        conv_w = np.asarray(inputs[p + "_conv_w"], np.float32)
        conv_b = np.asarray(inputs[p + "_conv_b"], np.float32)
        xproj_W = np.asarray(inputs[p + "_xproj_W"], np.float32)
        dt_W = np.asarray(inputs[p + "_dt_W"], np.float32)
        dt_b = np.asarray(inputs[p + "_dt_b"], np.float32)
        A_log = np.asarray(inputs[p + "_A_log"], np.float32)
        Dvec = np.asarray(inputs[p + "_D"], np.float32)
        out_W = np.asarray(inputs[p + "_out_W"], np.float32)
        proj_W = np.asarray(inputs["proj_W"], np.float32)

        # the kernel generates dA = exp(-n*dt); verify A has that structure
        A = -np.exp(A_log)
        assert np.allclose(A, -np.arange(1, N + 1, dtype=np.float32)[None, :]
                           .repeat(DI, 0), atol=1e-4), "unexpected A structure"

        own = slice(half * DH, (half + 1) * DH)
        xb = x[bat]
        if dire == 1:
            xb = xb[::-1]
        # channel order: own half first, then other half
        perm = np.concatenate([np.arange(half * DH, (half + 1) * DH),
                               np.arange((1 - half) * DH, (2 - half) * DH)])
        w_in_cat = np.concatenate([in_W[perm], in_W[DI + half * DH:DI + (half + 1) * DH]], 0)
        W_eff = proj_W[:, dire * DM:(dire + 1) * DM] @ out_W   # (DM, DI)

        # diagonal conv matrices: for block m, tap k -> diag(conv_w_perm[m*128:(m+1)*128, k])
        cw = conv_w[perm]                                       # (DI, 4)
        diag = np.zeros((NBLK_DF * K4 * 128, 128), np.float32)
        idx = np.arange(128)
        for m in range(NBLK_DF):
            for kk in range(K4):
                q = m * K4 + kk
                diag[q * 128 + idx, idx] = cw[m * 128 + idx, kk]

        m = {
            "xT": np.ascontiguousarray(xb.T).astype(bf),
            "w_in": np.ascontiguousarray(w_in_cat.T).astype(bf),
            "w_xp": np.ascontiguousarray(xproj_W[:, perm].T).astype(bf),
            "w_dt": np.ascontiguousarray(dt_W[own].T).astype(bf),
            "w_out": np.ascontiguousarray(W_eff[:, own].T).astype(bf),
            "conv_diag": np.ascontiguousarray(diag).astype(bf),
            "conv_b": np.ascontiguousarray(conv_b[perm][:, None]),
            "dt_b": np.ascontiguousarray(dt_b[own][:, None]),
            "Dv": np.ascontiguousarray(Dvec[own][:, None]),
        }
        maps.append(m)
    return maps


def _unshard(results, inputs):
    parts = [r["out"].astype(np.float32) for r in results]
    proj_b = np.asarray(inputs["proj_b"], np.float32)
    out = np.empty((B, L, DM), np.float32)
    for bat in range(2):
        fwd = parts[0 * 4 + bat * 2 + 0] + parts[0 * 4 + bat * 2 + 1]
        bwd = parts[1 * 4 + bat * 2 + 0] + parts[1 * 4 + bat * 2 + 1]
        out[bat] = (fwd + bwd[:, ::-1]).T + proj_b[None, :]
    return out


def kernel(**inputs):
    if "nc" not in _CACHED:
        _CACHED["nc"] = _build_module()
    nc = _CACHED["nc"]
    maps = _prep_inputs(inputs)
    res = bass_utils.run_bass_kernel_spmd(nc, maps, core_ids=list(range(8)))
    return _unshard(res.results, inputs)


# revision 16
# speedup vs baseline: 1.0570x; 1.0570x over previous
"""BiMamba Trainium2 kernel (8 NeuronCores, SPMD).

Sharding: core = dir(2) x batch(2) x d_inner-half(2).
Each core runs one direction's mamba block on one batch element for half of
d_inner. The xproj (which contracts over full d_inner) is handled by having
every core compute the full xi/conv/silu (cheap duplication) so no cross-core
communication is needed. The final out-proj + concat + output projection are
algebraically folded into one matmul with W_eff = proj_W[:, dir] @ out_W_dir;
each core emits a partial (d_model, L) which the host sums across the 4 cores
of each batch element.

v3 engine plan (from trace + microbench):
- depthwise conv on PE: 4 diagonal-matrix matmuls into PSUM; in-proj chunks
  carry a 3-column overlap so no halo copies are needed.
- selective scan: native tensor_tensor_scan on DVE, chained over two
  time-halves so phase B's first half overlaps phase A's last chunks
  (emission interleaved; engine streams are in-order).
- dA = exp(-n*dt) on the scalar engine.
- d1 = bsc*B always on DVE (it feeds the scan); ch = h*C mostly on GpSimd.
- y2 accumulation over the 16 states via GpSimd-issued accumulate-DMAs
  (SBUF->SBUF bf16) running on the DMA engines.
"""

import sys

sys.path.insert(0, "/opt/trn_rl_repo")

import numpy as np
import ml_dtypes

import concourse.bass as bass
import concourse.bacc as bacc
import concourse.mybir as mybir
import concourse.tile as tile
from concourse import bass_utils

F32 = mybir.dt.float32
BF16 = mybir.dt.bfloat16
AF = mybir.ActivationFunctionType
ALU = mybir.AluOpType

B, L, DM = 2, 2048, 1024
DI = 2048            # d_inner
DH = DI // 2         # per-core half of d_inner
N = 16               # d_state
R = 64               # dt_rank
K4 = 4               # d_conv
TC = 512             # time chunk for matmul phases
NCHUNK = L // TC
NBLK_DM = DM // 128      # 8 k-blocks over d_model
NBLK_DH = DH // 128      # 8 blocks over own half
NBLK_DF = DI // 128      # 16 blocks over full d_inner
LH = L // 2              # phase-B half length

_CACHED = {}


def _build_module():
    nc = bacc.Bacc("TRN2", target_bir_lowering=False, debug=False, num_devices=8)

    def din(name, shape, dt):
        return nc.dram_tensor(name, list(shape), dt, kind="ExternalInput").ap()

    xT = din("xT", (DM, L), BF16)                 # x (possibly flipped).T
    w_in = din("w_in", (DM, DI + DH), BF16)       # lhsT: [xi_own|xi_oth|z_own]
    w_xp = din("w_xp", (DI, 2 * N + R), BF16)     # lhsT for xproj (rows reordered)
    w_dt = din("w_dt", (R, DH), BF16)             # lhsT for dt proj (own half)
    w_out = din("w_out", (DH, DM), BF16)          # lhsT: W_eff own-half rows
    conv_diag = din("conv_diag", (NBLK_DF * K4 * 128, 128), BF16)  # diag conv mats
    conv_b = din("conv_b", (DI, 1), F32)
    dt_b = din("dt_b", (DH, 1), F32)
    Dv = din("Dv", (DH, 1), F32)
    out_d = nc.dram_tensor("out", [DM, L], F32, kind="ExternalOutput").ap()
    z_spill = nc.dram_tensor("z_spill", [DH, L], BF16, kind="Internal").ap()
    xc_spill = nc.dram_tensor("xc_spill", [DH, L], BF16, kind="Internal").ap()
    bc_spill = nc.dram_tensor("bc_spill", [2 * N, L], BF16, kind="Internal").ap()
    y2_spill = nc.dram_tensor("y2_spill", [DH, L], BF16, kind="Internal").ap()

    with tile.TileContext(nc) as tc:
        _emit(nc, tc, xT, w_in, w_xp, w_dt, w_out, conv_diag, conv_b, dt_b, Dv,
              out_d, z_spill, xc_spill, bc_spill, y2_spill)
    nc.compile()
    return nc


def _emit(nc, tc, xT, w_in, w_xp, w_dt, w_out, conv_diag, conv_b, dt_b, Dv,
          out_d, z_spill, xc_spill, bc_spill, y2_spill):
    from contextlib import ExitStack
    ctx = ExitStack()
    with ctx:
        # ---------------- persistent weights/consts ----------------
        wpool = ctx.enter_context(tc.tile_pool(name="weights", bufs=1))
        conv_b_sb = wpool.tile([128, NBLK_DF], F32, tag="conv_b", name="conv_b")
        nc.sync.dma_start(conv_b_sb[:],
                          conv_b.rearrange("(k p) c -> p k c", p=128))
        dt_b_sb = wpool.tile([128, NBLK_DH], F32, tag="dt_b", name="dt_b")
        nc.sync.dma_start(dt_b_sb[:],
                          dt_b.rearrange("(k p) c -> p k c", p=128))
        Dv_sb = wpool.tile([128, NBLK_DH], F32, tag="Dv", name="Dv")
        nc.sync.dma_start(Dv_sb[:],
                          Dv.rearrange("(k p) c -> p k c", p=128))

        # ---------------- resident activations ----------------
        rpool = ctx.enter_context(tc.tile_pool(name="resident", bufs=1))
        dt_own = [rpool.tile([128, L], BF16, tag=f"dt{b}", name=f"dt{b}")
                  for b in range(NBLK_DH)]
        bsc = [rpool.tile([128, L], BF16, tag=f"bsc{b}", name=f"bsc{b}")
               for b in range(NBLK_DH)]
        # chunk-boundary scan states: one [128, 1] column per (n, b)
        hs_pool = ctx.enter_context(tc.tile_pool(name="hstate", bufs=1))
        hstate = hs_pool.tile([128, N * NBLK_DH], F32, tag="hstate", name="hstate")

        # phase-B rotating pool must outlive (so open before) the phase-A pools
        bpool = ctx.enter_context(tc.tile_pool(name="phaseB", bufs=2))

        # ================= Phase A emitters =================
        actx = ExitStack()
        apw = actx.enter_context(tc.tile_pool(name="phaseA_w", bufs=1))
        apool = actx.enter_context(tc.tile_pool(name="phaseA", bufs=1))
        apsum = actx.enter_context(tc.tile_pool(name="phaseA_ps", bufs=2,
                                                space="PSUM"))
        cpsum = actx.enter_context(tc.tile_pool(name="phaseA_cps", bufs=2,
                                                space="PSUM"))
        ppsum = actx.enter_context(tc.tile_pool(name="phaseA_pps", bufs=1,
                                                space="PSUM"))
        mpool = actx.enter_context(tc.tile_pool(name="phaseA_misc", bufs=1))
        xi_sb = [mpool.tile([128, 3 + TC], BF16, tag=f"xi{m}", name=f"xi{m}")
                 for m in range(NBLK_DF)]

        w_in_sb = []
        for k in range(NBLK_DM):
            t = apw.tile([128, DI + DH], BF16, tag=f"w_in{k}", name=f"w_in{k}")
            nc.sync.dma_start(t[:], w_in[k * 128:(k + 1) * 128, :])
            w_in_sb.append(t)
        w_xp_sb = []
        for k in range(NBLK_DF):
            t = apw.tile([128, 2 * N + R], BF16, tag=f"w_xp{k}", name=f"w_xp{k}")
            nc.sync.dma_start(t[:], w_xp[k * 128:(k + 1) * 128, :])
            w_xp_sb.append(t)
        w_dt_sb = apw.tile([R, DH], BF16, tag="w_dt", name="w_dt")
        nc.sync.dma_start(w_dt_sb[:], w_dt[:, :])

        def emit_A_chunk(c):
            t0 = c * TC
            x_sb = []
            for k in range(NBLK_DM):
                t = apool.tile([128, TC], BF16, tag=f"x{k}", name=f"x{k}")
                nc.sync.dma_start(t[:],
                                  xT[k * 128:(k + 1) * 128, t0:t0 + TC])
                x_sb.append(t)
            xc_chunk = []
            for m in range(NBLK_DF + NBLK_DH):   # 16 xi blocks + 8 z blocks
                if m < NBLK_DF:
                    ps = apsum.tile([128, TC], F32, tag="inproj", name="inproj")
                    for k in range(NBLK_DM):
                        nc.tensor.matmul(ps[:],
                                         w_in_sb[k][:, m * 128:(m + 1) * 128],
                                         x_sb[k][:], start=(k == 0),
                                         stop=(k == NBLK_DM - 1))
                    xi = xi_sb[m]
                    if c == 0:
                        nc.vector.memset(xi[:, 0:3], 0.0)
                    else:
                        # save last 3 cols of previous chunk as the new halo
                        nc.scalar.activation(xi[:, 0:3], xi[:, TC:TC + 3], AF.Copy)
                    nc.scalar.activation(xi[:, 3:3 + TC], ps[:], AF.Copy)
                    cdg = apool.tile([128, K4 * 128], BF16, tag="cdiag",
                                     name="cdiag")
                    nc.sync.dma_start(
                        cdg[:],
                        conv_diag[m * K4 * 128:(m + 1) * K4 * 128, :]
                        .rearrange("(q p) j -> p q j", p=128))
                    cps = cpsum.tile([128, TC], F32, tag="convps", name="convps")
                    for kk in range(K4):
                        nc.tensor.matmul(cps[:],
                                         cdg[:, kk * 128:(kk + 1) * 128],
                                         xi[:, kk:kk + TC],
                                         start=(kk == 0), stop=(kk == K4 - 1))
                    xc_t = mpool.tile([128, TC], BF16, tag=f"xct{m}",
                                      name=f"xct{m}")
                    nc.scalar.activation(xc_t[:], cps[:], AF.Silu,
                                         bias=conv_b_sb[:, m:m + 1])
                    if m < NBLK_DH:
                        nc.sync.dma_start(
                            xc_spill[m * 128:(m + 1) * 128, t0:t0 + TC],
                            xc_t[:])
                    xc_chunk.append(xc_t)
                else:
                    ps = apsum.tile([128, TC], F32, tag="inproj", name="inproj")
                    for k in range(NBLK_DM):
                        nc.tensor.matmul(ps[:],
                                         w_in_sb[k][:, m * 128:(m + 1) * 128],
                                         x_sb[k][:], start=(k == 0),
                                         stop=(k == NBLK_DM - 1))
                    zb = m - NBLK_DF
                    zt = apool.tile([128, TC], BF16, tag="zt", name="zt")
                    nc.scalar.activation(zt[:], ps[:], AF.Silu)
                    nc.sync.dma_start(
                        z_spill[zb * 128:(zb + 1) * 128, t0:t0 + TC], zt[:])

            # xproj: (2N+R, TC) accumulated over 16 channel blocks
            ps96 = ppsum.tile([R + 2 * N, TC], F32, tag="xproj", name="xproj")
            for k in range(NBLK_DF):
                nc.tensor.matmul(ps96[:], w_xp_sb[k][:], xc_chunk[k][:],
                                 start=(k == 0), stop=(k == NBLK_DF - 1))
            xdbl = apool.tile([R + 2 * N, TC], BF16, tag="xdbl", name="xdbl")
            nc.scalar.activation(xdbl[:], ps96[:], AF.Copy)
            # B and C rows -> DRAM (bf16) for later broadcast-reload
            bcc = apool.tile([2 * N, TC], BF16, tag="bcc", name="bcc")
            nc.vector.tensor_copy(bcc[:], xdbl[R:R + 2 * N, :])
            nc.sync.dma_start(bc_spill[:, t0:t0 + TC], bcc[:])
            # dt proj + softplus, then bsc = dt * xc
            for mb in range(NBLK_DH):
                psd = ppsum.tile([128, TC], F32, tag="dtproj", name="dtproj")
                nc.tensor.matmul(psd[:], w_dt_sb[:, mb * 128:(mb + 1) * 128],
                                 xdbl[0:R, :], start=True, stop=True)
                spe = apool.tile([128, TC], F32, tag="spe", name="spe")
                nc.scalar.activation(spe[:], psd[:], AF.Exp,
                                     bias=dt_b_sb[:, mb:mb + 1])
                nc.scalar.activation(dt_own[mb][:, t0:t0 + TC], spe[:],
                                     AF.Ln, bias=1.0)
                nc.vector.tensor_tensor(bsc[mb][:, t0:t0 + TC],
                                        dt_own[mb][:, t0:t0 + TC],
                                        xc_chunk[mb][:], ALU.mult)

        # ================= Phase B emitters =================
        # static engine assignment for ch = h*C between DVE and GpSimd.
        # ~80/20 toward DVE measured best: heavier GpSimd use slows every
        # engine via SBUF contention.
        ch_on_v = set(i for i in range(N * NBLK_DH) if i % 5 != 2)

        def emit_B_half(hf, n):
            lo = hf * LH
            B_bc = bpool.tile([128, LH], BF16, tag="B_bc", name="B_bc")
            C_bc = bpool.tile([128, LH], BF16, tag="C_bc", name="C_bc")
            nc.sync.dma_start(
                B_bc[:], bc_spill[n:n + 1, lo:lo + LH].partition_broadcast(128))
            nc.sync.dma_start(
                C_bc[:], bc_spill[N + n:N + n + 1, lo:lo + LH]
                .partition_broadcast(128))
            for b in range(NBLK_DH):
                dA = bpool.tile([128, LH], BF16, tag="dA", name="dA")
                nc.scalar.activation(dA[:], dt_own[b][:, lo:lo + LH], AF.Exp,
                                     scale=-float(n + 1))
                d1 = bpool.tile([128, LH], BF16, tag="d1", name="d1")
                nc.vector.tensor_tensor(d1[:], bsc[b][:, lo:lo + LH], B_bc[:],
                                        ALU.mult)
                h = bpool.tile([128, LH], BF16, tag="h", name="h")
                sc = n * NBLK_DH + b
                if hf == 0:
                    nc.vector.tensor_tensor_scan(h[:], dA[:], d1[:], 0.0,
                                                 ALU.mult, ALU.add)
                    nc.scalar.activation(hstate[:, sc:sc + 1],
                                         h[:, LH - 1:LH], AF.Copy)
                else:
                    nc.vector.tensor_tensor_scan(h[:], dA[:], d1[:],
                                                 hstate[:, sc:sc + 1],
                                                 ALU.mult, ALU.add)
                ch = bpool.tile([128, LH], BF16, tag="ch", name="ch")
                if sc in ch_on_v:
                    nc.vector.tensor_tensor(ch[:], h[:], C_bc[:], ALU.mult)
                else:
                    nc.gpsimd.tensor_tensor(ch[:], h[:], C_bc[:], ALU.mult)
                dst = y2_spill[b * 128:(b + 1) * 128, lo:lo + LH]
                if n == 0:
                    nc.sync.dma_start(dst, ch[:])
                else:
                    nc.gpsimd.dma_start(dst, ch[:], accum_op=ALU.add)

        # ================= emission: interleave A and B1 =================
        emit_A_chunk(0)
        emit_A_chunk(1)
        for n in range(N):
            if n == 2:
                emit_A_chunk(2)
            if n == 7:
                emit_A_chunk(3)
            emit_B_half(0, n)
        actx.close()
        for n in range(N):
            emit_B_half(1, n)

        # ============= Phase C: gate + out-proj =============
        with tc.tile_pool(name="phaseC", bufs=2) as cpool, \
             tc.tile_pool(name="phaseC_ps", bufs=2, space="PSUM") as cpsum2, \
             tc.tile_pool(name="phaseC_s", bufs=1) as spool, \
             tc.tile_pool(name="phaseC_w", bufs=1) as cwpool:
            w_out_sb = []
            for k in range(NBLK_DH):
                t = cwpool.tile([128, DM], BF16, tag=f"w_out{k}", name=f"w_out{k}")
                nc.sync.dma_start(t[:], w_out[k * 128:(k + 1) * 128, :])
                w_out_sb.append(t)
            s_sb = []
            for b in range(NBLK_DH):
                xcr = cpool.tile([128, L], BF16, tag="xcr", name="xcr")
                nc.sync.dma_start(xcr[:], xc_spill[b * 128:(b + 1) * 128, :])
                zs = cpool.tile([128, L], BF16, tag="zs", name="zs")
                nc.sync.dma_start(zs[:], z_spill[b * 128:(b + 1) * 128, :])
                y2r = cpool.tile([128, L], BF16, tag="y2r", name="y2r")
                nc.sync.dma_start(y2r[:], y2_spill[b * 128:(b + 1) * 128, :])
                s = spool.tile([128, L], BF16, tag=f"s{b}", name=f"s{b}")
                xd = cpool.tile([128, L], BF16, tag="xd", name="xd")
                # s = (xcr*D + y2) * silu(z); the D-mult runs on the scalar
                # engine (per-partition scale), the rest on DVE
                nc.scalar.activation(xd[:], xcr[:], AF.Copy,
                                     scale=Dv_sb[:, b:b + 1])
                nc.vector.tensor_tensor(s[:], xd[:], y2r[:], ALU.add)
                nc.vector.tensor_tensor(s[:], s[:], zs[:], ALU.mult)
                s_sb.append(s)
            for m in range(NBLK_DM):
                for c in range(NCHUNK):
                    ps = cpsum2.tile([128, TC], F32, tag="oproj", name="oproj")
                    for k in range(NBLK_DH):
                        nc.tensor.matmul(
                            ps[:], w_out_sb[k][:, m * 128:(m + 1) * 128],
                            s_sb[k][:, c * TC:(c + 1) * TC],
                            start=(k == 0), stop=(k == NBLK_DH - 1))
                    ot = cpool.tile([128, TC], F32, tag="ot", name="ot")
                    nc.vector.tensor_copy(ot[:], ps[:])
                    nc.sync.dma_start(
                        out_d[m * 128:(m + 1) * 128, c * TC:(c + 1) * TC],
                        ot[:])


def _prep_inputs(inputs):
    """Build the 8 per-core input maps from full inputs (numpy fp32)."""
    bf = ml_dtypes.bfloat16
    x = np.asarray(inputs["x"], np.float32)
    maps = []
    for core in range(8):
        dire, bat, half = core // 4, (core // 2) % 2, core % 2
        p = "fwd" if dire == 0 else "bwd"
        in_W = np.asarray(inputs[p + "_in_W"], np.float32)
        conv_w = np.asarray(inputs[p + "_conv_w"], np.float32)
        conv_b = np.asarray(inputs[p + "_conv_b"], np.float32)
        xproj_W = np.asarray(inputs[p + "_xproj_W"], np.float32)
        dt_W = np.asarray(inputs[p + "_dt_W"], np.float32)
        dt_b = np.asarray(inputs[p + "_dt_b"], np.float32)
        A_log = np.asarray(inputs[p + "_A_log"], np.float32)
        Dvec = np.asarray(inputs[p + "_D"], np.float32)
        out_W = np.asarray(inputs[p + "_out_W"], np.float32)
        proj_W = np.asarray(inputs["proj_W"], np.float32)

        # the kernel generates dA = exp(-n*dt); verify A has that structure
        A = -np.exp(A_log)
        assert np.allclose(A, -np.arange(1, N + 1, dtype=np.float32)[None, :]
                           .repeat(DI, 0), atol=1e-4), "unexpected A structure"

        own = slice(half * DH, (half + 1) * DH)
        xb = x[bat]
        if dire == 1:
            xb = xb[::-1]
        # channel order: own half first, then other half
        perm = np.concatenate([np.arange(half * DH, (half + 1) * DH),
                               np.arange((1 - half) * DH, (2 - half) * DH)])
        w_in_cat = np.concatenate([in_W[perm], in_W[DI + half * DH:DI + (half + 1) * DH]], 0)
        W_eff = proj_W[:, dire * DM:(dire + 1) * DM] @ out_W   # (DM, DI)

        # diagonal conv matrices: for block m, tap k -> diag(conv_w_perm[m*128:(m+1)*128, k])
        cw = conv_w[perm]                                       # (DI, 4)
        diag = np.zeros((NBLK_DF * K4 * 128, 128), np.float32)
        idx = np.arange(128)
        for m in range(NBLK_DF):
            for kk in range(K4):
                q = m * K4 + kk
                diag[q * 128 + idx, idx] = cw[m * 128 + idx, kk]

        m = {
            "xT": np.ascontiguousarray(xb.T).astype(bf),
            "w_in": np.ascontiguousarray(w_in_cat.T).astype(bf),
            "w_xp": np.ascontiguousarray(xproj_W[:, perm].T).astype(bf),
            "w_dt": np.ascontiguousarray(dt_W[own].T).astype(bf),
            "w_out": np.ascontiguousarray(W_eff[:, own].T).astype(bf),
            "conv_diag": np.ascontiguousarray(diag).astype(bf),
            "conv_b": np.ascontiguousarray(conv_b[perm][:, None]),
            "dt_b": np.ascontiguousarray(dt_b[own][:, None]),
            "Dv": np.ascontiguousarray(Dvec[own][:, None]),
        }
        maps.append(m)
    return maps


def _unshard(results, inputs):
    parts = [r["out"].astype(np.float32) for r in results]
    proj_b = np.asarray(inputs["proj_b"], np.float32)
    out = np.empty((B, L, DM), np.float32)
    for bat in range(2):
        fwd = parts[0 * 4 + bat * 2 + 0] + parts[0 * 4 + bat * 2 + 1]
        bwd = parts[1 * 4 + bat * 2 + 0] + parts[1 * 4 + bat * 2 + 1]
        out[bat] = (fwd + bwd[:, ::-1]).T + proj_b[None, :]
    return out


def kernel(**inputs):
    if "nc" not in _CACHED:
        _CACHED["nc"] = _build_module()
    nc = _CACHED["nc"]
    maps = _prep_inputs(inputs)
    res = bass_utils.run_bass_kernel_spmd(nc, maps, core_ids=list(range(8)))
    return _unshard(res.results, inputs)
